# revision 2
# baseline (speedup 1.0000x reference)
"""VQ codebook lookup (nn_VQ) on 8 TRN2 NeuronCores.

reference: idx = argmin_k ||x_n - e_k||^2 ; out = embeddings[idx]
Equivalent: idx = argmax_k (x_n . e_k - 0.5||e_k||^2)

Strategy (data-parallel over N, codebook replicated; device computes only
block-maxima of approximate scores, host finishes the argmax exactly):
  - Host: shard x into 8 x [62500, 100], round to bf16, TRANSPOSE on host and
    pad to xT [104, 62976] (rows 0:100 = x_hi.T, rows 100:103 = 1.0 bias-aug).
    Codebook side: one [104, 104] bf16 matrix: cols k<100 hold e_hi[k,:] with
    three bf16 bias-split rows carrying -0.5||e_k||^2; pad cols score -100.
  - Device, per 512-row super-tile: one contiguous DMA load of xT[:, 512],
    4 matmuls [104,128]x[104,104] -> scores [128,4,104] f32 in PSUM,
    one DVE block-reduce max over groups of 8 -> [128,4,13], DMA out.
    Out traffic is 13 f32/row instead of 100 (3.3 MB/core vs 25 MB/core).
  - Host: picks the best block per row (f32), recomputes the exact winner
    inside that 8-wide block with dense per-block GEMMs, and fully
    recomputes rows whose top-2 block gap < tau (covers all bf16 score
    error), then gathers embeddings[idx].
"""

import sys

sys.path.insert(0, "/opt/trn_rl_repo")
from contextlib import ExitStack

import ml_dtypes
import numpy as np

import concourse.bass as bass
import concourse.bacc as bacc
import concourse.tile as tile
from concourse import mybir
from concourse._compat import with_exitstack
from concourse.bass_utils import run_bass_kernel_spmd

BF = mybir.dt.bfloat16
F32 = mybir.dt.float32
bf16 = ml_dtypes.bfloat16

N_TOTAL = 500_000
D = 100
K = 100
CT = 104  # contraction: 100 dims + 3 bias rows + 1 zero pad
KP = 104  # padded codebook columns (13 blocks of 8)
NB = 13  # blocks per row
BS = 8  # block size
N_CORES = 8
ST = 512  # rows per super-tile
N_SHARD = N_TOTAL // N_CORES  # 62500
N_ST = -(-N_SHARD // ST)  # 123 super-tiles
NP = N_ST * ST  # 62976 padded rows per core
TAU = 1.2e-2  # host re-check threshold on top-2 block gap


@with_exitstack
def _vq_tile_kernel(ctx: ExitStack, tc: tile.TileContext, out, xt_d, et_d):
    nc = tc.nc

    consts = ctx.enter_context(tc.tile_pool(name="consts", bufs=1))
    et_s = consts.tile([CT, KP], BF, tag="et")
    nc.sync.dma_start(et_s[:], et_d[:])

    xp = ctx.enter_context(tc.tile_pool(name="xt", bufs=4))
    sp = ctx.enter_context(tc.tile_pool(name="scores", bufs=4, space="PSUM"))
    op = ctx.enter_context(tc.tile_pool(name="best", bufs=4))

    out_v = out.rearrange("(t p) (c b) -> t p c b", p=128, c=4)

    for t in range(N_ST):
        xt = xp.tile([CT, ST], BF, tag="xt")
        nc.sync.dma_start(out=xt[:], in_=xt_d[:, bass.ts(t, ST)])
        scores = sp.tile([128, 4, KP], F32, tag="scores")
        for c in range(4):
            nc.tensor.matmul(
                scores[:, c], xt[:, bass.ts(c, 128)], et_s[:], start=True, stop=True
            )
        best = op.tile([128, 4, NB], F32, tag="best")
        sv = scores[:].rearrange("p c (b j) -> p c b j", j=BS)
        nc.vector.tensor_reduce(
            best[:], sv, axis=mybir.AxisListType.X, op=mybir.AluOpType.max
        )
        nc.sync.dma_start(out=out_v[t], in_=best[:])


def build_nc():
    nc = bacc.Bacc(
        "TRN2",
        target_bir_lowering=False,
        debug=False,
        enable_asserts=True,
        num_devices=N_CORES,
    )
    out = nc.dram_tensor("out", [N_ST * 128, 4 * NB], F32, kind="ExternalOutput").ap()
    xt_d = nc.dram_tensor("xt", [CT, NP], BF, kind="ExternalInput").ap()
    et_d = nc.dram_tensor("et", [CT, KP], BF, kind="ExternalInput").ap()
    with tile.TileContext(nc) as tc:
        _vq_tile_kernel(tc, out, xt_d, et_d)
    nc.compile()
    return nc


def prep_inputs(inputs: np.ndarray, embeddings: np.ndarray):
    """Host-side shard + layout prep. Returns in_maps for the 8 cores."""
    x = np.ascontiguousarray(inputs, dtype=np.float32)
    e = np.ascontiguousarray(embeddings, dtype=np.float32)

    e64 = e.astype(np.float64)
    b = -0.5 * np.sum(e64 * e64, axis=1)  # [K] exact bias
    e_hi = e.astype(bf16)
    b1 = (b).astype(bf16)
    b2 = (b - b1.astype(np.float64)).astype(bf16)
    b3 = (b - b1.astype(np.float64) - b2.astype(np.float64)).astype(bf16)

    et = np.zeros((CT, KP), dtype=bf16)
    et[0:D, 0:K] = e_hi.T
    et[100, 0:K] = b1
    et[101, 0:K] = b2
    et[102, 0:K] = b3
    et[100, K:KP] = bf16(-100.0)  # pad codes can never win

    x_hi = x.astype(bf16)

    in_maps = []
    for i in range(N_CORES):
        lo_r, hi_r = i * N_SHARD, (i + 1) * N_SHARD
        xt = np.zeros((CT, NP), dtype=bf16)
        xt[0:D, :N_SHARD] = x_hi[lo_r:hi_r].T
        xt[100:103, :N_SHARD] = bf16(1.0)
        in_maps.append({"xt": xt, "et": et})
    return in_maps


def postprocess(bm: np.ndarray, x: np.ndarray, e: np.ndarray) -> np.ndarray:
    """bm: [N_TOTAL, NB] f32 device block-maxima. Returns embeddings[idx]."""
    n = bm.shape[0]
    b1i = np.argmax(bm, axis=1)
    part = np.partition(bm, NB - 2, axis=1)
    gap = part[:, NB - 1] - part[:, NB - 2]
    flag = gap < TAU

    x64 = x.astype(np.float64)
    e64 = e.astype(np.float64)
    bias = -0.5 * np.sum(e64 * e64, axis=1)

    idx = np.empty(n, dtype=np.int64)
    for blk in range(NB):
        m = (b1i == blk) & ~flag
        if not m.any():
            continue
        k0 = blk * BS
        ks = np.arange(k0, min(k0 + BS, K))
        sc = x64[m] @ e64[ks].T + bias[ks][None, :]
        idx[m] = k0 + sc.argmax(axis=1)
    if flag.any():
        sc = x64[flag] @ e64.T + bias[None, :]
        idx[flag] = sc.argmax(axis=1)
    return np.ascontiguousarray(e[idx], dtype=np.float32)


_NC_CACHE = None


def kernel(inputs: np.ndarray, embeddings: np.ndarray) -> np.ndarray:
    global _NC_CACHE
    if _NC_CACHE is None:
        _NC_CACHE = build_nc()
    nc = _NC_CACHE
    in_maps = prep_inputs(inputs, embeddings)
    res = run_bass_kernel_spmd(nc, in_maps, core_ids=list(range(N_CORES)))
    shards = []
    for i in range(N_CORES):
        o = res.results[i]["out"].reshape(N_ST, 128, 4, NB)
        o = o.transpose(0, 2, 1, 3).reshape(NP, NB)[:N_SHARD]
        shards.append(o)
    bm = np.concatenate(shards, axis=0)
    return postprocess(bm, np.asarray(inputs, dtype=np.float32), np.asarray(embeddings, dtype=np.float32))


# revision 3
# speedup vs baseline: 2.3392x; 2.3392x over previous
"""VQ codebook lookup (nn_VQ) on 8 TRN2 NeuronCores.

reference: idx = argmin_k ||x_n - e_k||^2 ; out = embeddings[idx]
Equivalent: idx = argmax_k (x_n . e_k - 0.5||e_k||^2)

Strategy (data-parallel over N, codebook replicated; device computes only
block-maxima of approximate scores, host finishes the argmax exactly):
  - Host: shard x into 8 x [62500, 100], round to bf16, TRANSPOSE on host and
    pad to xT [104, 65536] (rows 0:100 = x_hi.T, rows 100:103 = 1.0 bias-aug).
    Codebook side: one [104, 104] bf16 matrix: cols k<100 hold e_hi[k,:] with
    three bf16 bias-split rows carrying -0.5||e_k||^2; pad cols score -100.
  - Device, per 4096-row batch (8 super-tiles of 512): one contiguous DMA load
    of xT[:, 4096] (8 KB/partition -> few, large descriptors; DGE was the
    bottleneck of the naive version), then per super-tile: 4 matmuls
    [104,128]x[104,104] -> scores [128,4,104] f32 in PSUM, one DVE block-max
    reduce over groups of 8 -> [128,4,13]; batch results staged in SBUF and
    stored with one DMA per batch on the second HWDGE queue (scalar/Act).
    Out traffic is 13 f32/row instead of 100 (3.4 MB/core vs 25 MB/core).
  - Host: picks the best block per row (f32), recomputes the exact winner
    inside that 8-wide block with dense per-block GEMMs, and fully
    recomputes rows whose top-2 block gap < tau (covers all bf16 score
    error), then gathers embeddings[idx].
"""

import sys

sys.path.insert(0, "/opt/trn_rl_repo")
from contextlib import ExitStack

import ml_dtypes
import numpy as np

import concourse.bass as bass
import concourse.bacc as bacc
import concourse.tile as tile
from concourse import mybir
from concourse._compat import with_exitstack
from concourse.bass_utils import run_bass_kernel_spmd

BF = mybir.dt.bfloat16
F32 = mybir.dt.float32
bf16 = ml_dtypes.bfloat16

N_TOTAL = 500_000
D = 100
K = 100
CT = 104  # contraction: 100 dims + 3 bias rows + 1 zero pad
KP = 104  # padded codebook columns (13 blocks of 8)
NB = 13  # blocks per row
BS = 8  # block size
N_CORES = 8
ST = 512  # rows per super-tile (one PSUM bank of scores)
BT = 8  # super-tiles per DMA batch
N_SHARD = N_TOTAL // N_CORES  # 62500
N_ST = 128  # super-tiles per core (padded)
N_BT = N_ST // BT  # 16 batches
NP = N_ST * ST  # 65536 padded rows per core
TAU = 1.2e-2  # host re-check threshold on top-2 block gap


@with_exitstack
def _vq_tile_kernel(ctx: ExitStack, tc: tile.TileContext, out, xt_d, et_d):
    nc = tc.nc

    consts = ctx.enter_context(tc.tile_pool(name="consts", bufs=1))
    et_s = consts.tile([CT, KP], BF, tag="et")
    nc.sync.dma_start(et_s[:], et_d[:])

    xp = ctx.enter_context(tc.tile_pool(name="xt", bufs=3))
    sp = ctx.enter_context(tc.tile_pool(name="scores", bufs=4, space="PSUM"))
    op = ctx.enter_context(tc.tile_pool(name="best", bufs=3))

    for bt in range(N_BT):
        xt = xp.tile([CT, BT * ST], BF, tag="xt")
        nc.sync.dma_start(out=xt[:], in_=xt_d[:, bass.ts(bt, BT * ST)])
        btile = op.tile([128, BT, 4, NB], F32, tag="best")
        for s in range(BT):
            scores = sp.tile([128, 4, KP], F32, tag="scores")
            for c in range(4):
                nc.tensor.matmul(
                    scores[:, c],
                    xt[:, bass.ts(s * 4 + c, 128)],
                    et_s[:],
                    start=True,
                    stop=True,
                )
            sv = scores[:].rearrange("p c (b j) -> p c b j", j=BS)
            nc.vector.tensor_reduce(
                btile[:, s], sv, axis=mybir.AxisListType.X, op=mybir.AluOpType.max
            )
        nc.scalar.dma_start(out=out[:, bass.ts(bt, BT * 4 * NB)], in_=btile[:])


def build_nc():
    nc = bacc.Bacc(
        "TRN2",
        target_bir_lowering=False,
        debug=False,
        enable_asserts=True,
        num_devices=N_CORES,
    )
    out = nc.dram_tensor("out", [128, N_ST * 4 * NB], F32, kind="ExternalOutput").ap()
    xt_d = nc.dram_tensor("xt", [CT, NP], BF, kind="ExternalInput").ap()
    et_d = nc.dram_tensor("et", [CT, KP], BF, kind="ExternalInput").ap()
    with tile.TileContext(nc) as tc:
        _vq_tile_kernel(tc, out, xt_d, et_d)
    nc.compile()
    return nc


def prep_inputs(inputs: np.ndarray, embeddings: np.ndarray):
    """Host-side shard + layout prep. Returns in_maps for the 8 cores."""
    x = np.ascontiguousarray(inputs, dtype=np.float32)
    e = np.ascontiguousarray(embeddings, dtype=np.float32)

    e64 = e.astype(np.float64)
    b = -0.5 * np.sum(e64 * e64, axis=1)  # [K] exact bias
    e_hi = e.astype(bf16)
    b1 = (b).astype(bf16)
    b2 = (b - b1.astype(np.float64)).astype(bf16)
    b3 = (b - b1.astype(np.float64) - b2.astype(np.float64)).astype(bf16)

    et = np.zeros((CT, KP), dtype=bf16)
    et[0:D, 0:K] = e_hi.T
    et[100, 0:K] = b1
    et[101, 0:K] = b2
    et[102, 0:K] = b3
    et[100, K:KP] = bf16(-100.0)  # pad codes can never win

    x_hi = x.astype(bf16)

    in_maps = []
    for i in range(N_CORES):
        lo_r, hi_r = i * N_SHARD, (i + 1) * N_SHARD
        xt = np.zeros((CT, NP), dtype=bf16)
        xt[0:D, :N_SHARD] = x_hi[lo_r:hi_r].T
        xt[100:103, :N_SHARD] = bf16(1.0)
        in_maps.append({"xt": xt, "et": et})
    return in_maps


def postprocess(bm: np.ndarray, x: np.ndarray, e: np.ndarray) -> np.ndarray:
    """bm: [N_TOTAL, NB] f32 device block-maxima. Returns embeddings[idx]."""
    n = bm.shape[0]
    b1i = np.argmax(bm, axis=1)
    part = np.partition(bm, NB - 2, axis=1)
    gap = part[:, NB - 1] - part[:, NB - 2]
    flag = gap < TAU

    x64 = x.astype(np.float64)
    e64 = e.astype(np.float64)
    bias = -0.5 * np.sum(e64 * e64, axis=1)

    idx = np.empty(n, dtype=np.int64)
    for blk in range(NB):
        m = (b1i == blk) & ~flag
        if not m.any():
            continue
        k0 = blk * BS
        ks = np.arange(k0, min(k0 + BS, K))
        sc = x64[m] @ e64[ks].T + bias[ks][None, :]
        idx[m] = k0 + sc.argmax(axis=1)
    if flag.any():
        sc = x64[flag] @ e64.T + bias[None, :]
        idx[flag] = sc.argmax(axis=1)
    return np.ascontiguousarray(e[idx], dtype=np.float32)


_NC_CACHE = None


def kernel(inputs: np.ndarray, embeddings: np.ndarray) -> np.ndarray:
    global _NC_CACHE
    if _NC_CACHE is None:
        _NC_CACHE = build_nc()
    nc = _NC_CACHE
    in_maps = prep_inputs(inputs, embeddings)
    res = run_bass_kernel_spmd(nc, in_maps, core_ids=list(range(N_CORES)))
    shards = []
    for i in range(N_CORES):
        o = res.results[i]["out"].reshape(128, N_ST, 4, NB)  # [p, t, c, b]
        o = o.transpose(1, 2, 0, 3).reshape(NP, NB)[:N_SHARD]
        shards.append(o)
    bm = np.concatenate(shards, axis=0)
    return postprocess(
        bm, np.asarray(inputs, dtype=np.float32), np.asarray(embeddings, dtype=np.float32)
    )
